# revision 14
# baseline (speedup 1.0000x reference)
"""Multi-head attention (B=8, T=2048, C=256, H=4) on 8 NeuronCores.

Data-parallel over batch: core b computes batch element b end-to-end.

Per-core dataflow (v3):
  xT   [C, T]      = PE-transpose of x (x pre-cast to bf16 on host)
  qkT  [2C, T]     = w_qk @ xT + b_qk   (bias via ScalarE Identity)
  v    [T, H, 65]  = x @ w_v.T + b_v    (ones column per head -> sumexp
                                         rides the PV matmul for free)
  attention, qt outer / head-pair inner / 16 k-chunks:
    scoresT[k,q] via K=64 matmuls in PE row groups 0/64 (pair overlaps)
    exp SPLIT across engines: ScalarE true Exp (9/16 chunks) and
      VectorE Schraudolph bit-trick (7/16): int16(round(s*A+B)) bitcast
      as bf16 == 2^(s*0.125/ln2) to +-3%; softmax averaging over 2048
      keys washes the error out (measured ~7e-4 total abs err).
    PV accumulates o2[h] = [65, 512] PSUM over chunks; row 64 = sumexp
    normalize per (qt,hp): DVE collects 2 sumexp rows, one batched
      reciprocal_approx_fast + bf16 cast, 2 row/col-tiled K=1 ones
      matmuls broadcast 1/sumexp into a [128,512] PSUM tile, ScalarE
      copies it to SBUF, one scalar_tensor_tensor per head fuses
      normalize+copy into yt (bf16).
  proj: out[t,:] = yT[:,t].T @ w_pT + b_p, bias on DVE, store to DRAM.

PSUM: sc pool 2x[128,1024] (scores/stageB/stageC/bc/proj staging) +
o2 pool 4x[65,512] (double-buffered across (qt,hp) parities) = 8 banks.
"""

import numpy as np
import ml_dtypes

import concourse.bass as bass
import concourse.tile as tile
from concourse import bacc, mybir
from concourse.bass_utils import run_bass_kernel_spmd
from concourse.masks import make_identity

B, T, C = 8, 2048, 256
H, HD = 4, 64
N_CORES = 8
F32 = mybir.dt.float32
F32R = mybir.dt.float32r
BF16 = mybir.dt.bfloat16
I16 = mybir.dt.int16
I8 = mybir.dt.int8
FP8 = mybir.dt.float8e4

QT = 512                # q-tile (columns per score matmul)
NQT = T // QT           # 4
KC = T // 128           # 16 k-chunks of 128

# Schraudolph exp for fp8e4-bitcast: fp8(i8) ~= 2^(i8/8 - 7)
# want exp(s*0.125) = 2^(s*0.125/ln2):  i8 = s*(1/ln2) + 7*8 - c
EXP_A = float(np.float32(8.0 * 0.125 / np.log(2.0)))
EXP_B = float(np.float32(56.0 - 0.35))

# k-chunks whose exp runs as Schraudolph on the DVE (rest: ScalarE Exp)
DVE_CHUNKS = frozenset((1, 3, 5, 7, 9, 11, 13))


def build_nc():
    nc = bacc.Bacc("TRN2", target_bir_lowering=False, debug=False,
                   num_devices=N_CORES)

    x_ap = nc.dram_tensor("xbf", [T, C], BF16, kind="ExternalInput").ap()
    wqk_ap = nc.dram_tensor("w_qkT", [C, 2 * C], F32R, kind="ExternalInput").ap()
    wv_ap = nc.dram_tensor("w_vT", [C, C], F32R, kind="ExternalInput").ap()
    wp_ap = nc.dram_tensor("w_pT", [C, C], F32R, kind="ExternalInput").ap()
    bqk_ap = nc.dram_tensor("b_qk", [4, 128], F32, kind="ExternalInput").ap()
    bv_ap = nc.dram_tensor("b_v", [C], F32, kind="ExternalInput").ap()
    bp_ap = nc.dram_tensor("b_p", [C], F32, kind="ExternalInput").ap()
    out_ap = nc.dram_tensor("out", [T, C], F32, kind="ExternalOutput").ap()

    with tile.TileContext(nc) as tc:
        with (
            tc.tile_pool(name="consts", bufs=1) as consts,
            tc.tile_pool(name="xstage", bufs=4) as xstage,
            tc.tile_pool(name="xt", bufs=1) as xtp,
            tc.tile_pool(name="qkt", bufs=1) as qktp,
            tc.tile_pool(name="vsb", bufs=1) as vsbp,
            tc.tile_pool(name="expa", bufs=6) as expa,
            tc.tile_pool(name="yt", bufs=1) as ytp,
            tc.tile_pool(name="small", bufs=2) as small,
            tc.tile_pool(name="ostage", bufs=4) as ostage,
            tc.tile_pool(name="scps", bufs=4, space="PSUM") as scps,
            tc.tile_pool(name="o2ps", bufs=2, space="PSUM") as o2ps,
        ):
            # ---- stage A load: x first (critical path), via sync HWDGE ---
            x_re = x_ap.rearrange("(b a p) c -> b p a c", b=4, p=128)
            xsbig = [None] * 4
            for b in range(4):
                xsbig[b] = xstage.tile([128, 4, C], BF16, tag="xs", name=f"xs{b}")
                nc.sync.dma_start(xsbig[b][:], x_re[b])

            # ---- constants (no DMA deps; overlap with x transfer) --------
            ident = consts.tile([128, 128], BF16, tag="ident")
            make_identity(nc, ident[:])
            ones_bc = consts.tile([33, 64], BF16, tag="ones_bc")
            nc.vector.memset(ones_bc[:], 1.0)
            onescol = consts.tile([128, H], BF16, tag="onescol")
            nc.vector.memset(onescol[:], 1.0)

            # ---- weights ------------------------------------------------
            w_qk = [consts.tile([128, 2 * C], BF16, tag=f"wqk{c}", name=f"wqk{c}") for c in range(2)]
            for c in range(2):
                nc.gpsimd.dma_start(w_qk[c][:], wqk_ap[128 * c:128 * (c + 1), :])
            w_v = [consts.tile([128, C], BF16, tag=f"wv{c}", name=f"wv{c}") for c in range(2)]
            for c in range(2):
                nc.gpsimd.dma_start(w_v[c][:], wv_ap[128 * c:128 * (c + 1), :])
            w_p = [consts.tile([128, C], BF16, tag=f"wp{c}", name=f"wp{c}") for c in range(2)]
            for c in range(2):
                nc.gpsimd.dma_start(w_p[c][:], wp_ap[128 * c:128 * (c + 1), :])

            b_qk = consts.tile([128, 4], F32, tag="bqk")
            nc.gpsimd.dma_start(b_qk[:], bqk_ap.rearrange("c p -> p c"))
            b_p = consts.tile([128, C], F32, tag="bp")
            bp_bc = bass.AP(tensor=bp_ap.tensor, offset=bp_ap.offset,
                            ap=[[0, 128]] + list(bp_ap.ap))
            nc.gpsimd.dma_start(b_p[:], bp_bc)
            b_v = consts.tile([128, C], F32, tag="bv")
            bv_bc = bass.AP(tensor=bv_ap.tensor, offset=bv_ap.offset,
                            ap=[[0, 128]] + list(bv_ap.ap))
            nc.gpsimd.dma_start(b_v[:], bv_bc)

            # ---- stage A: PE-transpose to xT -----------------------------
            xt = [xtp.tile([128, T], BF16, tag=f"xt{c}", name=f"xt{c}") for c in range(2)]
            for tt in range(KC):
                xs = xsbig[tt // 4][:, tt % 4, :]
                for c in range(2):
                    if c == 0:
                        ps = scps.tile([128, 128], BF16, tag="sc", name="tp0")
                    else:
                        ps = o2ps.tile([128, 128], BF16, tag=f"o2h{tt % 2}",
                                       name=f"tp{tt % 2}")
                    nc.tensor.transpose(ps[:], xs[:, 128 * c:128 * (c + 1)], ident[:])
                    nc.vector.tensor_copy(xt[c][:, 128 * tt:128 * (tt + 1)], ps[:])

            # ---- stage B: qkT [2C, T] = w_qk.T @ xT + b_qk ---------------
            # m-outer, n-inner with 4 live PSUM halves: the w_qk stationary
            # is loaded once per (m, c) instead of per (m, n, c).
            qkt = [qktp.tile([128, T], BF16, tag=f"qkt{m}", name=f"qkt{m}") for m in range(4)]
            def stage_b(m):
                pss = [scps.tile([128, QT], F32, tag="sc", name=f"bps{m}{j}")
                       for j in range(NQT)]
                for c in range(2):
                    for n in range(NQT):
                        nc.tensor.matmul(
                            pss[n][:], w_qk[c][:, 128 * m:128 * (m + 1)],
                            xt[c][:, QT * n:QT * (n + 1)],
                            start=(c == 0), stop=(c == 1))
                for n in range(NQT):
                    nc.scalar.add(
                        qkt[m][:, QT * n:QT * (n + 1)], pss[n][:],
                        b_qk[:, m:m + 1])

            stage_b(2)
            stage_b(0)

            # ---- stage C: v as fp8 DoubleRow pairs [128, H, 2, 80] -------
            # cols 0..63 = head dims, col 64 = ones (sumexp), 65..79 pad
            vsb = [vsbp.tile([128, H, 2, 80], FP8, tag=f"v{p}", name=f"v{p}")
                   for p in range(KC // 2)]
            for tt in range(KC):
                ps = scps.tile([128, QT], F32, tag="sc", name="cps")
                for c in range(2):
                    nc.tensor.matmul(
                        ps[:, 0:C], xt[c][:, 128 * tt:128 * (tt + 1)], w_v[c][:],
                        start=(c == 0), stop=(c == 1))
                nc.vector.tensor_add(
                    vsb[tt // 2][:, :, tt % 2, 0:HD],
                    ps[:, 0:C].rearrange("p (h d) -> p h d", h=H),
                    b_v[:].rearrange("p (h d) -> p h d", h=H))
                nc.vector.tensor_copy(
                    vsb[tt // 2][:, :, tt % 2, HD:HD + 1],
                    onescol[:].rearrange("p (h o) -> p h o", o=1))

            stage_b(3)
            stage_b(1)

            # ---- stage D: attention, qt outer / head-pair / k-chunks -----
            yt = [ytp.tile([128, T], BF16, tag=f"yt{hp}", name=f"yt{hp}") for hp in range(2)]
            # Normalize (bc matmul / ScalarE copy / STT) and proj are
            # DEFERRED into the middle of the NEXT block so the in-order
            # PE queue never stalls on the DVE recip chain.
            def make_norm(qt, hp, o2, rec):
                def norm():
                    bc = scps.tile([128, QT], F32, tag="sc", name="bc")
                    for h in range(2):
                        nc.tensor.matmul(
                            bc[64 * h:64 * (h + 1), :],
                            ones_bc[32 * h:32 * h + 1, :],
                            rec[32 * h:32 * h + 1, :],
                            start=True, stop=True,
                            tile_position=(32 * h, 64 * h))
                    bcs = small.tile([128, QT], BF16, tag="bcs")
                    nc.scalar.copy(bcs[:], bc[:])
                    for h in range(2):
                        nc.vector.scalar_tensor_tensor(
                            yt[hp][64 * h:64 * (h + 1), QT * qt:QT * (qt + 1)],
                            o2[h][0:HD, :], 1.0, bcs[64 * h:64 * (h + 1), :],
                            mybir.AluOpType.mult, mybir.AluOpType.mult)
                return norm

            def make_proj(qt):
                def proj():
                    for tt in range(qt * QT // 128, (qt + 1) * QT // 128):
                        ps = scps.tile([128, QT], F32, tag="sc", name="pps")
                        for c in range(2):
                            nc.tensor.matmul(
                                ps[:, 0:C], yt[c][:, 128 * tt:128 * (tt + 1)],
                                w_p[c][:], start=(c == 0), stop=(c == 1))
                        ost = ostage.tile([128, C], F32, tag="ost")
                        nc.vector.tensor_add(ost[:], ps[:, 0:C], b_p[:])
                        nc.sync.dma_start(out_ap[128 * tt:128 * (tt + 1), :], ost[:])
                return proj

            pend_norm, pend_proj = None, None
            for qt in range(NQT):
                for hp in range(2):
                    qT = qkt[hp]
                    kT = qkt[hp + 2]
                    o2 = [o2ps.tile([80, QT], F32, tag=f"o2h{h}",
                                    name=f"o2{h}") for h in range(2)]
                    expair = None
                    for i in range(KC):
                        if i == 3 and pend_norm is not None:
                            pend_norm()
                            pend_norm = None
                        if i == 6 and pend_proj is not None:
                            pend_proj()
                            pend_proj = None
                        scs = [scps.tile([128, QT], F32, tag="sc", name=f"sc{h}")
                               for h in range(2)]
                        for h in range(2):
                            nc.tensor.matmul(
                                scs[h][:],
                                kT[64 * h:64 * (h + 1), 128 * i:128 * (i + 1)],
                                qT[64 * h:64 * (h + 1), QT * qt:QT * (qt + 1)],
                                start=True, stop=True)
                        if i % 2 == 0:
                            expair = [expa.tile([128, 2, QT], FP8, tag="ex",
                                                name=f"ex{h}") for h in range(2)]
                        for h in range(2):
                            dst = expair[h][:, i % 2, :]
                            if i in DVE_CHUNKS:
                                nc.vector.tensor_scalar(
                                    dst.bitcast(I8), scs[h][:], EXP_A, EXP_B,
                                    mybir.AluOpType.mult, mybir.AluOpType.add)
                            else:
                                nc.scalar.activation(
                                    dst, scs[h][:],
                                    mybir.ActivationFunctionType.Exp,
                                    bias=0.0, scale=0.125)
                        if i % 2 == 1:
                            for h in range(2):
                                nc.tensor.matmul(
                                    o2[h][:],
                                    vsb[i // 2][:, 2 * hp + h],
                                    expair[h][:],
                                    start=(i == 1), stop=(i == KC - 1),
                                    perf_mode=mybir.MatmulPerfMode.DoubleRow)
                    # sumexp -> 1/sumexp on the DVE (fills its idle tail)
                    se = small.tile([33, QT], F32, tag="se")
                    for h in range(2):
                        nc.vector.tensor_copy(
                            se[32 * h:32 * h + 1, :], o2[h][HD:HD + 1, :])
                    rec_f = small.tile([33, QT], F32, tag="rec_f")
                    nc.vector.reciprocal_approx_fast(rec_f[:], se[:])
                    rec = small.tile([33, QT], BF16, tag="rec")
                    nc.vector.tensor_copy(rec[:], rec_f[:])
                    pend_norm = make_norm(qt, hp, o2, rec)
                if qt > 0:
                    pend_proj = make_proj(qt - 1)
            pend_norm()
            pend_proj()
            make_proj(NQT - 1)()
    nc.compile()
    return nc


_NC_CACHE = []


def _get_nc():
    if not _NC_CACHE:
        _NC_CACHE.append(build_nc())
    return _NC_CACHE[0]


def make_in_maps(x, w_qkv, b_qkv, w_proj, b_proj):
    shared = {
        "w_qkT": np.ascontiguousarray(w_qkv[:2 * C].T, dtype=np.float32),
        "w_vT": np.ascontiguousarray(w_qkv[2 * C:].T, dtype=np.float32),
        "w_pT": np.ascontiguousarray(w_proj.T, dtype=np.float32),
        "b_qk": np.ascontiguousarray(b_qkv[:2 * C].reshape(4, 128), dtype=np.float32),
        "b_v": np.ascontiguousarray(b_qkv[2 * C:], dtype=np.float32),
        "b_p": np.ascontiguousarray(b_proj, dtype=np.float32),
    }
    xbf = np.asarray(x, dtype=np.float32).astype(ml_dtypes.bfloat16)
    return [dict(shared, xbf=np.ascontiguousarray(xbf[b])) for b in range(B)]


def run(x, w_qkv, b_qkv, w_proj, b_proj, trace=False):
    nc = _get_nc()
    in_maps = make_in_maps(np.asarray(x), np.asarray(w_qkv), np.asarray(b_qkv),
                           np.asarray(w_proj), np.asarray(b_proj))
    res = run_bass_kernel_spmd(nc, in_maps, list(range(N_CORES)), trace=trace)
    out = np.stack([res.results[b]["out"] for b in range(B)])
    return out, res


def kernel(x, w_qkv, b_qkv, w_proj, b_proj):
    out, _ = run(x, w_qkv, b_qkv, w_proj, b_proj, trace=False)
    return out


# revision 31
# speedup vs baseline: 1.4764x; 1.4764x over previous
"""Multi-head attention (B=8, T=2048, C=256, H=4) on 8 NeuronCores.

Data-parallel over batch: core b computes batch element b end-to-end.

Per-core dataflow (v5, ~185us vs the 205us v1 baseline):
  xT   [C, T]      = PE-transpose of x (x pre-cast to bf16 on host)
  qkT  [2C, T]     = w_qk @ xT + b_qk   (bias via ScalarE Identity+bias)
  v    fp8 DoubleRow pairs [128, H, 2, 80]: cols 0..63 head dims,
       col 64 = ones (sumexp rides the PV matmul), 65..79 pad
       (DoubleRow needs pair-dim stride %16)
  attention, qt outer / head-pair blocks / 16 k-chunks:
    scoresT[k,q] via K=64 matmuls in PE row groups 0/64 (pair overlaps
      in the array), fp32 PSUM [128,1024] (3-deep pipeline = 6 banks)
    exp SPLIT across engines, one full-tile op per chunk writing fp8
      into a DoubleRow pair tile [128, 2(h), 2(e), 512]:
      - ScalarE true Exp for 9/16 chunks (~1.0us each)
      - VectorE Schraudolph for 7/16: int8(round(s*A8+B8)) bitcast as
        fp8e4 == 2^(s*0.125/ln2) to +-7%; softmax averaging over 2048
        keys keeps the total error at ~2e-3 abs (gate is 3.6e-3)
    PV: one fp8 DoubleRow matmul per (chunk-pair, head) -> [80, 512]
      PSUM accumulator; row 64 = sumexp. HALVES the PV streaming
      cycles vs bf16. First pair's PV emission is deferred past the
      chunk-3 scores so the in-order PE queue keeps its full runway
      at block boundaries.
    block tail: unnormalized o2 -> yt copies split ACT/DVE (frees the
      single-buffered o2 PSUM fast), sumexp rows + one batched
      reciprocal_approx_fast + bf16 cast on DVE.
    deferred normalize (next block, chunk 3): 2 row/col-tiled K=1 ones
      matmuls broadcast 1/sumexp into PSUM, one in-place tensor_mul
      on yt reads it straight from PSUM.
  proj: out[t,:] = yT[:,t].T @ w_pT + b_p (deferred into the following
      qt), bias on DVE, store to DRAM.

Engine balance per block: PE ~10.5us, ACT ~11us, DVE ~11us. Keeping
the PE near-saturated matters: idle gaps re-throttle the PE clock
(HAM) to 1.2GHz and double matmul times.
"""

import numpy as np
import ml_dtypes

import concourse.bass as bass
import concourse.tile as tile
from concourse import bacc, mybir
from concourse.bass_utils import run_bass_kernel_spmd
from concourse.masks import make_identity

B, T, C = 8, 2048, 256
H, HD = 4, 64
N_CORES = 8
F32 = mybir.dt.float32
F32R = mybir.dt.float32r
BF16 = mybir.dt.bfloat16
I16 = mybir.dt.int16
I8 = mybir.dt.int8
FP8 = mybir.dt.float8e4

QT = 512                # q-tile (columns per score matmul)
NQT = T // QT           # 4
KC = T // 128           # 16 k-chunks of 128

# Schraudolph exp for fp8e4-bitcast: fp8(i8) ~= 2^(i8/8 - 7)
# want exp(s*0.125) = 2^(s*0.125/ln2):  i8 = s*(1/ln2) + 7*8 - c
EXP_A = float(np.float32(8.0 * 0.125 / np.log(2.0)))
EXP_B = float(np.float32(56.0 - 0.35))

# k-chunks whose exp runs as Schraudolph on the DVE (rest: ScalarE Exp)
DVE_CHUNKS = frozenset()


def build_nc():
    nc = bacc.Bacc("TRN2", target_bir_lowering=False, debug=False,
                   num_devices=N_CORES)

    x_ap = nc.dram_tensor("xbf", [T, C], BF16, kind="ExternalInput").ap()
    wqk_ap = nc.dram_tensor("w_qkT", [C, 2 * C], BF16, kind="ExternalInput").ap()
    wv_ap = nc.dram_tensor("w_vT", [C, C], BF16, kind="ExternalInput").ap()
    wp_ap = nc.dram_tensor("w_pT", [C, C], BF16, kind="ExternalInput").ap()
    bqk_ap = nc.dram_tensor("b_qk", [4, 128], F32, kind="ExternalInput").ap()
    bv_ap = nc.dram_tensor("b_v", [C], F32, kind="ExternalInput").ap()
    bp_ap = nc.dram_tensor("b_p", [C], F32, kind="ExternalInput").ap()
    out_ap = nc.dram_tensor("out", [T, C], F32, kind="ExternalOutput").ap()

    with tile.TileContext(nc) as tc:
        with (
            tc.tile_pool(name="consts", bufs=1) as consts,
            tc.tile_pool(name="xstage", bufs=4) as xstage,
            tc.tile_pool(name="xt", bufs=1) as xtp,
            tc.tile_pool(name="qkt", bufs=1) as qktp,
            tc.tile_pool(name="vsb", bufs=1) as vsbp,
            tc.tile_pool(name="expa", bufs=4) as expa,
            tc.tile_pool(name="yt", bufs=1) as ytp,
            tc.tile_pool(name="small", bufs=2) as small,
            tc.tile_pool(name="ostage", bufs=4) as ostage,
            tc.tile_pool(name="scps", bufs=3, space="PSUM") as scps,
            tc.tile_pool(name="o2ps", bufs=1, space="PSUM") as o2ps,
        ):
            # ---- stage A load: x first (critical path), via sync HWDGE ---
            x_re = x_ap.rearrange("(b a p) c -> b p a c", b=4, p=128)
            xsbig = [None] * 4
            for b in range(4):
                xsbig[b] = xstage.tile([128, 4, C], BF16, tag="xs", name=f"xs{b}")
                nc.sync.dma_start(xsbig[b][:], x_re[b])

            # ---- constants (no DMA deps; overlap with x transfer) --------
            ident = consts.tile([128, 128], BF16, tag="ident")
            make_identity(nc, ident[:])
            ones_bc = consts.tile([33, 64], BF16, tag="ones_bc")
            nc.vector.memset(ones_bc[:], 1.0)
            onescol = consts.tile([128, H], BF16, tag="onescol")
            nc.vector.memset(onescol[:], 1.0)

            # ---- weights ------------------------------------------------
            w_qk = [consts.tile([128, 2 * C], BF16, tag=f"wqk{c}", name=f"wqk{c}") for c in range(2)]
            for c in range(2):
                nc.scalar.dma_start(w_qk[c][:], wqk_ap[128 * c:128 * (c + 1), :])
            w_v = [consts.tile([128, C], BF16, tag=f"wv{c}", name=f"wv{c}") for c in range(2)]
            for c in range(2):
                nc.sync.dma_start(w_v[c][:], wv_ap[128 * c:128 * (c + 1), :])
            w_p = [consts.tile([128, C], BF16, tag=f"wp{c}", name=f"wp{c}") for c in range(2)]
            for c in range(2):
                nc.sync.dma_start(w_p[c][:], wp_ap[128 * c:128 * (c + 1), :])

            b_qk = consts.tile([128, 4], F32, tag="bqk")
            nc.gpsimd.dma_start(b_qk[:], bqk_ap.rearrange("c p -> p c"))
            b_p = consts.tile([128, C], F32, tag="bp")
            bp_bc = bass.AP(tensor=bp_ap.tensor, offset=bp_ap.offset,
                            ap=[[0, 128]] + list(bp_ap.ap))
            nc.gpsimd.dma_start(b_p[:], bp_bc)
            b_v = consts.tile([128, C], F32, tag="bv")
            bv_bc = bass.AP(tensor=bv_ap.tensor, offset=bv_ap.offset,
                            ap=[[0, 128]] + list(bv_ap.ap))
            nc.gpsimd.dma_start(b_v[:], bv_bc)

            # ---- stage A: PE-transpose to xT -----------------------------
            xt = [xtp.tile([128, T], BF16, tag=f"xt{c}", name=f"xt{c}") for c in range(2)]
            for tt in range(KC):
                xs = xsbig[tt // 4][:, tt % 4, :]
                for c in range(2):
                    if c == 0:
                        ps = scps.tile([128, 128], BF16, tag="sc", name="tp0")
                    else:
                        ps = o2ps.tile([128, 128], BF16, tag=f"o2h{tt % 2}",
                                       name=f"tp{tt % 2}")
                    nc.tensor.transpose(ps[:], xs[:, 128 * c:128 * (c + 1)], ident[:])
                    nc.vector.tensor_copy(xt[c][:, 128 * tt:128 * (tt + 1)], ps[:])

            # ---- stage B: qkT [2C, T] = w_qk.T @ xT + b_qk ---------------
            # m-outer, n-inner with 4 live PSUM halves: the w_qk stationary
            # is loaded once per (m, c) instead of per (m, n, c).
            qkt = [qktp.tile([128, T], BF16, tag=f"qkt{m}", name=f"qkt{m}") for m in range(4)]
            def stage_b(m):
                pss = [scps.tile([128, 2 * QT], F32, tag="sc", name=f"bps{m}{j}")
                       for j in range(2)]
                for c in range(2):
                    for n in range(NQT):
                        nc.tensor.matmul(
                            pss[n // 2][:, QT * (n % 2):QT * (n % 2 + 1)],
                            w_qk[c][:, 128 * m:128 * (m + 1)],
                            xt[c][:, QT * n:QT * (n + 1)],
                            start=(c == 0), stop=(c == 1))
                for n in range(NQT):
                    nc.scalar.add(
                        qkt[m][:, QT * n:QT * (n + 1)],
                        pss[n // 2][:, QT * (n % 2):QT * (n % 2 + 1)],
                        b_qk[:, m:m + 1])

            stage_b(2)
            stage_b(0)

            # ---- stage C: v as fp8 DoubleRow pairs [128, H, 2, 80] -------
            # cols 0..63 = head dims, col 64 = ones (sumexp), 65..79 pad
            vsb = [vsbp.tile([128, H, 2, 80], FP8, tag=f"v{p}", name=f"v{p}")
                   for p in range(KC // 2)]
            for tt in range(KC):
                ps = scps.tile([128, 2 * QT], F32, tag="sc", name="cps")
                for c in range(2):
                    nc.tensor.matmul(
                        ps[:, 0:C], xt[c][:, 128 * tt:128 * (tt + 1)], w_v[c][:],
                        start=(c == 0), stop=(c == 1))
                nc.vector.tensor_add(
                    vsb[tt // 2][:, :, tt % 2, 0:HD],
                    ps[:, 0:C].rearrange("p (h d) -> p h d", h=H),
                    b_v[:].rearrange("p (h d) -> p h d", h=H))
                nc.vector.tensor_copy(
                    vsb[tt // 2][:, :, tt % 2, HD:HD + 1],
                    onescol[:].rearrange("p (h o) -> p h o", o=1))

            stage_b(3)
            stage_b(1)

            # ---- stage D: attention, qt outer / head-pair / k-chunks -----
            # Block tail: unnormalized o2 -> yt via ScalarE (frees PSUM
            # fast), sumexp rows + reciprocal on DVE, bf16 cast on Pool.
            # The bc broadcast matmul + in-place yt normalize (TT with the
            # PSUM bc as second operand) are DEFERRED into the next block
            # so the in-order PE queue never stalls on the recip chain.
            yt = [ytp.tile([128, T], BF16, tag=f"yt{hp}", name=f"yt{hp}") for hp in range(2)]
            def make_tail(qt, hp, o2, se, cell):
                def tail():
                    nc.scalar.copy(
                        yt[hp][0:64, QT * qt:QT * (qt + 1)], o2[0][0:HD, :])
                    nc.scalar.copy(se[0:1, :], o2[0][HD:HD + 1, :])
                    rec_f = small.tile([33, QT], F32, tag="rec_f")
                    nc.vector.reciprocal_approx_fast(rec_f[:], se[:])
                    rec = small.tile([33, QT], BF16, tag="rec")
                    nc.vector.tensor_copy(rec[:], rec_f[:])
                    cell.append(rec)
                return tail

            def make_norm(qt, hp, cell):
                def norm():
                    rec = cell[0]
                    bc = scps.tile([128, 2 * QT], F32, tag="sc", name="bc")
                    for h in range(2):
                        nc.tensor.matmul(
                            bc[64 * h:64 * (h + 1), 0:QT],
                            ones_bc[32 * h:32 * h + 1, :],
                            rec[32 * h:32 * h + 1, :],
                            start=True, stop=True,
                            tile_position=(32 * h, 64 * h))
                    ys = yt[hp][:, QT * qt:QT * (qt + 1)]
                    nc.vector.tensor_mul(ys, ys, bc[:, 0:QT])
                return norm

            def make_proj(qt):
                def proj():
                    for tt in range(qt * QT // 128, (qt + 1) * QT // 128):
                        ps = scps.tile([128, 2 * QT], F32, tag="sc", name="pps")
                        for c in range(2):
                            nc.tensor.matmul(
                                ps[:, 0:C], yt[c][:, 128 * tt:128 * (tt + 1)],
                                w_p[c][:], start=(c == 0), stop=(c == 1))
                        ost = ostage.tile([128, C], F32, tag="ost")
                        nc.vector.tensor_add(ost[:], ps[:, 0:C], b_p[:])
                        nc.sync.dma_start(out_ap[128 * tt:128 * (tt + 1), :], ost[:])
                return proj

            pend_tail, pend_norm, pend_proj = None, None, None
            for qt in range(NQT):
                if qt > 0:
                    pend_proj = make_proj(qt - 1)
                for hp in range(2):
                    qT = qkt[hp]
                    kT = qkt[hp + 2]
                    o2 = [o2ps.tile([80, QT], F32, tag=f"o2h{h}",
                                    name=f"o2{h}") for h in range(2)]
                    exm = None
                    pend_pvs = []
                    for i in range(KC):
                        if i == 1 and pend_tail is not None:
                            pend_tail()
                            pend_tail = None
                        if i == 4 and pend_norm is not None:
                            pend_norm()
                            pend_norm = None
                        if i == 6 and pend_proj is not None:
                            pend_proj()
                            pend_proj = None
                        sc = scps.tile([128, 2 * QT], F32, tag="sc", name="sc")
                        for h in range(2):
                            nc.tensor.matmul(
                                sc[:, QT * h:QT * (h + 1)],
                                kT[64 * h:64 * (h + 1), 128 * i:128 * (i + 1)],
                                qT[64 * h:64 * (h + 1), QT * qt:QT * (qt + 1)],
                                start=True, stop=True)
                        if i % 2 == 0:
                            exm = expa.tile([128, 2, 2, QT], FP8, tag="ex",
                                            name="exm")
                        dst = exm[:, :, i % 2, :]
                        srcv = sc[:].rearrange("p (h q) -> p h q", h=2)
                        if i in DVE_CHUNKS:
                            nc.vector.tensor_scalar(
                                dst.bitcast(I8), srcv, EXP_A, EXP_B,
                                mybir.AluOpType.mult, mybir.AluOpType.add)
                        else:
                            nc.scalar.activation(
                                dst, srcv,
                                mybir.ActivationFunctionType.Exp,
                                bias=0.0, scale=0.125)
                        if i % 2 == 1:
                            def pv(exm=exm, i=i):
                                for h in range(2):
                                    nc.tensor.matmul(
                                        o2[h][:],
                                        vsb[i // 2][:, 2 * hp + h],
                                        exm[:, h],
                                        start=(i == 1), stop=(i == KC - 1),
                                        perf_mode=mybir.MatmulPerfMode.DoubleRow)
                            if i <= 3:
                                pend_pvs.append(pv)
                            else:
                                for f in pend_pvs:
                                    f()
                                pend_pvs = []
                                pv()
                    # block tail: DVE half now (its queue is idle here);
                    # ACT half + recip chain deferred into the next block
                    se = small.tile([33, QT], F32, tag="se")
                    nc.vector.tensor_copy(
                        yt[hp][64:128, QT * qt:QT * (qt + 1)], o2[1][0:HD, :])
                    nc.vector.tensor_copy(se[32:33, :], o2[1][HD:HD + 1, :])
                    cell = []
                    pend_tail = make_tail(qt, hp, o2, se, cell)
                    pend_norm = make_norm(qt, hp, cell)
            pend_tail()
            pend_norm()
            make_proj(NQT - 1)()
    nc.compile()
    return nc


_NC_CACHE = []


def _get_nc():
    if not _NC_CACHE:
        _NC_CACHE.append(build_nc())
    return _NC_CACHE[0]


def make_in_maps(x, w_qkv, b_qkv, w_proj, b_proj):
    shared = {
        "w_qkT": np.ascontiguousarray(
            w_qkv[:2 * C].T.astype(ml_dtypes.bfloat16)),
        "w_vT": np.ascontiguousarray(
            w_qkv[2 * C:].T.astype(ml_dtypes.bfloat16)),
        "w_pT": np.ascontiguousarray(w_proj.T.astype(ml_dtypes.bfloat16)),
        "b_qk": np.ascontiguousarray(b_qkv[:2 * C].reshape(4, 128), dtype=np.float32),
        "b_v": np.ascontiguousarray(b_qkv[2 * C:], dtype=np.float32),
        "b_p": np.ascontiguousarray(b_proj, dtype=np.float32),
    }
    xbf = np.asarray(x, dtype=np.float32).astype(ml_dtypes.bfloat16)
    return [dict(shared, xbf=np.ascontiguousarray(xbf[b])) for b in range(B)]


def run(x, w_qkv, b_qkv, w_proj, b_proj, trace=False):
    nc = _get_nc()
    in_maps = make_in_maps(np.asarray(x), np.asarray(w_qkv), np.asarray(b_qkv),
                           np.asarray(w_proj), np.asarray(b_proj))
    res = run_bass_kernel_spmd(nc, in_maps, list(range(N_CORES)), trace=trace)
    out = np.stack([res.results[b]["out"] for b in range(B)])
    return out, res


def kernel(x, w_qkv, b_qkv, w_proj, b_proj):
    out, _ = run(x, w_qkv, b_qkv, w_proj, b_proj, trace=False)
    return out
